# revision 23
# baseline (speedup 1.0000x reference)
"""Trainium2 Bass kernel for nn_Attend (l2-distance attention with zero-kv).

Reference computation (per b,h):
    k' = [0; k], v' = [0; v]                       (prepend zero kv)
    scores[i,j] = (2 q_i.k'_j - |q_i|^2 - |k'_j|^2) * (D+2)^-0.5
    causal: j <= i+1 in padded index space
    out = softmax(scores) @ v'

Kernel algebra: softmax is invariant to the per-row constant -scale*|q_i|^2,
so with p~[i,j] = exp(2*scale*q_i.k_j) and ek_j = exp(-scale*|k_j|^2) folded
into the PV stationary operand [V*ek | ek] (zero column contributes exp(0)=1
to the denominator only):
    out_i = (sum_j p~ (v_j ek_j)) / (1 + sum_j p~ ek_j)

Layout: scores are computed TRANSPOSED ([kv, q]); heads are processed in
PAIRS, with the two heads' QK matmuls row-tiled onto PE halves (base
partitions 0/64) so they run CONCURRENTLY.

PV uses P^T 128x128 chunks as the STATIONARY operand and [V*ek | ek]
[128, 65] as the MOVING operand, accumulating out[q, 0:65] per q-chunk in
PSUM across kv blocks.  This puts the softmax denominator in PSUM column
64 PER PARTITION (q), so finalize is a tiny DVE chain (add 1, reciprocal,
broadcast multiply) with no PE transposes, no activation-table switches,
and the output leaves the device in natural [q, d] layout.

exp is split across two engines to break the ACT bottleneck:
  - ACT: activation Exp (diagonal blocks + ~half the off-diagonal blocks)
  - DVE: Schraudolph bf16 exp: i16 = trunc(s*C1M + C2P) bit-cast to bf16
    approximates exp(2*scale*s) to ~1.8% rms; one tensor_scalar per block.
Causal masking touches only the 128-col mixed band of each diagonal block
(GPSIMD multiply); QK/exp/PV are column-restricted past the band.

Host-side prep (make_in_maps): bf16 cast + transposed input layouts + the
[V*ek | ek] PV operand (ek computed on host from bf16-rounded k) + mask
constants.

Sharding: 32 (b,h) pairs -> 4 heads per core, 8 cores, pure data parallel.
"""

import sys

for _p in ("/opt/trn_rl_repo", "/root/.axon_site"):
    if _p not in sys.path:
        sys.path.insert(0, _p)

import numpy as np

B, H, N, D = 2, 16, 2048, 64
NCORES = 8
HPC = (B * H) // NCORES          # heads per core = 4
NPAIRS = HPC // 2
SCALE = float((D + 2) ** -0.5)   # augmented head dim, matches reference
NB = N // 128                    # kv blocks of 128 = 16
NQT = N // 512                   # q tiles of 512 = 4
LOG2E = 1.4426950408889634
C1M = float(2.0 * SCALE * 128.0 * LOG2E)
CSH = 0.0580                     # schraudolph correction (tuned, floor conv)
C2P = float(16256.0 - 128.0 * CSH + 0.5)  # +0.5: int16 convert truncates

_BUILT = {}


def _build(qk_dt="bfloat16", pv_dt="bfloat16", hpc=HPC, n=N):
    """Build + finalize the SPMD Bass program (one core's view)."""
    NB = n // 128
    NQT = n // 512
    import concourse.mybir as mybir
    import concourse.tile as tile
    from concourse import bacc

    f32 = mybir.dt.float32
    bf16 = mybir.dt.bfloat16
    i16 = mybir.dt.int16
    Exp = mybir.ActivationFunctionType.Exp
    add = mybir.AluOpType.add
    mult = mybir.AluOpType.mult

    nc = bacc.Bacc("TRN2", target_bir_lowering=False, debug=False, num_swdge_queues=4)
    qtp_p = nc.declare_dram_parameter("qtp", [NPAIRS, 128, n], bf16, isOutput=False)
    kt2_p = nc.declare_dram_parameter("kt2", [NPAIRS, 128, n], bf16, isOutput=False)
    vo_p = nc.declare_dram_parameter("vo", [hpc, 128, NB, 65], bf16, isOutput=False)
    mg_p = nc.declare_dram_parameter("mg", [128, 2, 128], bf16, isOutput=False)
    o_p = nc.declare_dram_parameter("out", [hpc, n, 64], f32, isOutput=True)

    # off-diagonal exp engine schedule: alternate DVE/ACT (tunable ratio)

    with tile.TileContext(nc) as tc:
        with (
            tc.tile_pool(name="const", bufs=1) as constp,
            tc.tile_pool(name="kqt", bufs=2) as kqtp,
            tc.tile_pool(name="vop", bufs=4) as vop,
            tc.tile_pool(name="pt", bufs=6) as ptp,
            tc.tile_pool(name="fin", bufs=3) as finp,
            tc.tile_pool(name="ps_s", bufs=3, space="PSUM") as ps_s,
            tc.tile_pool(name="ps_acc", bufs=2, space="PSUM") as ps_acc,
        ):
            mg = constp.tile([128, 2, 128], bf16, tag="mg")

            # ---- load all pairs (ek pre-folded into vo on host) -----
            # pair0's q/k staged in 512-col leading chunks (exactly what
            # QK t=0 consumes) so the first matmul starts ~10us in; mg
            # rides the scalar queue between kt2 chunks (needed by the
            # first diag mask, right after the first exp).
            qTps, kT2s, vos = [], [], {}
            for pair in range(NPAIRS):
                hA, hB = 2 * pair, 2 * pair + 1
                qTp = kqtp.tile([128, n], bf16, tag="qTp", name=f"qTp_{pair}")
                kT2 = kqtp.tile([128, n], bf16, tag="kT2", name=f"kT2_{pair}")
                if pair == 0:
                    # QK j=0 needs kT2 cols 0:128 + qTp cols 0:512 only;
                    # land those first (qtp's 512 split across two queues)
                    nc.sync.dma_start(out=qTp[:, 0:256], in_=qtp_p[pair][:, 0:256])
                    nc.gpsimd.dma_start(
                        out=qTp[:, 256:512], in_=qtp_p[pair][:, 256:512]
                    )
                    for a, b in ((0, 128), (128, 512), (512, 1024), (1024, n)):
                        nc.scalar.dma_start(out=kT2[:, a:b], in_=kt2_p[pair][:, a:b])
                        if a == 0:
                            nc.scalar.dma_start(out=mg[:], in_=mg_p[:])
                    nc.sync.dma_start(
                        out=qTp[:, 512:1024], in_=qtp_p[pair][:, 512:1024]
                    )
                    nc.sync.dma_start(out=qTp[:, 1024:n], in_=qtp_p[pair][:, 1024:n])
                else:
                    nc.sync.dma_start(out=qTp[:], in_=qtp_p[pair])
                    nc.scalar.dma_start(out=kT2[:], in_=kt2_p[pair])
                qTps.append(qTp)
                kT2s.append(kT2)
                for h in (hA, hB):
                    vos[h] = vop.tile(
                        [128, NB, 65], bf16, tag="vo", name=f"vo_{h}"
                    )
                if pair == 0:
                    # both heads' first 4 kv blocks land first (t=0 PV)
                    for h in (hA, hB):
                        nc.gpsimd.dma_start(
                            out=vos[h][:, 0:4], in_=vo_p[h][:, 0:4]
                        )
                    for h in (hA, hB):
                        nc.gpsimd.dma_start(
                            out=vos[h][:, 4:NB], in_=vo_p[h][:, 4:NB]
                        )
                else:
                    for h in (hA, hB):
                        nc.gpsimd.dma_start(out=vos[h][:], in_=vo_p[h])

            # ---- main flash loop ------------------------------------
            # greedy ACT/DVE balance for exp (diag blocks eligible for
            # DVE-Schraudolph too); finalize DVE ops are deferred and
            # drip-fed between blocks so they never burst-serialize the
            # DVE queue at a tile boundary.
            act_load = 0.0
            dve_load = 0.0
            fin_q = []
            pvq = []
            for pair in range(NPAIRS):
                hA, hB = 2 * pair, 2 * pair + 1
                qTp, kT2 = qTps[pair], kT2s[pair]
                voA, voB = vos[hA], vos[hB]

                # pair1 runs tiles big-first so the pair boundary meets a
                # dense 16-block tile (keeps the PE HAM-warm through it)
                t_order = range(NQT) if pair == 0 else range(NQT - 1, -1, -1)
                for t in t_order:
                    nblk = 4 * (t + 1)
                    # per-head accumulators: [q-chunk part, 4 chunks, V|den]
                    # padded to a full PSUM bank so the single start=True
                    # (whole-bank has_written clear) owns the bank.
                    accT = [
                        ps_acc.tile(
                            [128, 4, 65],
                            f32,
                            tag="acc",
                            name=f"ac{pair}_{t}_{h2}",
                            padded_shape=[128, 4, 128],
                        )
                        for h2 in range(2)
                    ]

                    # PV (stationary = P^T chunk, moving = vo) is deferred
                    # by 4 blocks ACROSS tile boundaries so the PE FIFO
                    # always has real work during each tile's QK/exp
                    # warmup.  Finalize pops wait until the previous
                    # tile's PVs have fully drained (j >= 4) and run
                    # BEFORE this tile's first PV so the acc-slot
                    # write-after-read order is correct.
                    def make_pv(tt, jj, ptj, accTs, voAB):
                        def emit():
                            rr = jj - 4 * tt
                            for h2 in range(2):
                                for qc in range(max(rr, 0), 4):
                                    nc.tensor.matmul(
                                        accTs[h2][:, qc, :],
                                        ptj[
                                            :,
                                            512 * h2 + 128 * qc : 512 * h2
                                            + 128 * (qc + 1),
                                        ],
                                        voAB[h2][:, jj, :],
                                        start=(jj == 0 and qc == 0),
                                        stop=(jj == 4 * tt + qc),
                                    )

                        return emit

                    for j in range(nblk):
                        r = j - 4 * t
                        diag = r >= 0
                        c0 = 128 * r if diag else 0  # column restriction
                        qsA = qTp[0:64, 512 * t + c0 : 512 * (t + 1)]
                        qsB = qTp[64:128, 512 * t + c0 : 512 * (t + 1)]
                        sp = ps_s.tile([128, 1024], f32, tag="sp")
                        nc.tensor.matmul(
                            sp[:, c0:512],
                            kT2[0:64, 128 * j : 128 * (j + 1)],
                            qsA,
                            start=True,
                            stop=True,
                        )
                        nc.tensor.matmul(
                            sp[:, 512 + c0 : 1024],
                            kT2[64:128, 128 * j : 128 * (j + 1)],
                            qsB,
                            start=True,
                            stop=True,
                        )
                        if len(pvq) >= 4:
                            pvq.pop(0)[1]()
                        # all older-tile PVs drained -> safe to emit the
                        # older tiles' finalize before this tile's first PV
                        if fin_q and (not pvq or pvq[0][0] == (pair, t)):
                            while fin_q:
                                fin_q.pop(0)()
                        pt = ptp.tile([128, 1024], bf16, tag="pt")
                        sps = sp[:].rearrange("p (h c) -> p h c", h=2)[:, :, c0:512]
                        pts = pt[:].rearrange("p (h c) -> p h c", h=2)[:, :, c0:512]
                        w = 2 * (512 - c0)  # free-dim per partition
                        ca = (172.0 + w) / 1.2
                        cd = (120.0 + w) / 0.96
                        use_dve = (dve_load + cd) < (act_load + ca)
                        if use_dve:
                            dve_load += cd
                            nc.vector.tensor_scalar(
                                pts.bitcast(i16), sps, C1M, C2P, mult, add
                            )
                        else:
                            act_load += ca
                            nc.scalar.activation(
                                pts, sps, Exp, scale=2.0 * SCALE
                            )
                        if diag:
                            # mask the 128-wide mixed band of both heads
                            band = pt[:].rearrange("p (h c) -> p h c", h=2)[
                                :, :, c0 : c0 + 128
                            ]
                            nc.gpsimd.tensor_tensor(band, band, mg[:], mult)
                        pvq.append(
                            ((pair, t), make_pv(t, j, pt, accT, (voA, voB)))
                        )

                    # ---- finalize: per-partition den -> tiny DVE chain,
                    # emitted lazily (2 ops per subsequent block)
                    def make_fin(pair, t, h2, h, accTs):
                        def fin_a():
                            rec = finp.tile(
                                [128, 4, 1],
                                f32,
                                tag="rec",
                                name=f"rc{pair}_{t}_{h2}",
                            )
                            nc.vector.tensor_scalar_add(
                                rec[:, :, 0], accTs[:, :, 64], 1.0
                            )
                            nc.vector.reciprocal(rec[:], rec[:])
                            st["rec"] = rec

                        def fin_b():
                            nrm = finp.tile(
                                [128, 4, 64],
                                f32,
                                tag="nrm",
                                name=f"nr{pair}_{t}_{h2}",
                            )
                            recb = st["rec"][:].broadcast_to([128, 4, 64])
                            nc.vector.scalar_tensor_tensor(
                                nrm[:], accTs[:, :, 0:64], 1.0, recb, mult, mult
                            )
                            nc.sync.dma_start(
                                out=o_p[h][512 * t : 512 * (t + 1), :].rearrange(
                                    "(c p) d -> p c d", p=128
                                ),
                                in_=nrm[:],
                            )

                        st = {}
                        return [fin_a, fin_b]

                    for h2, h in enumerate((hA, hB)):
                        fin_q.extend(make_fin(pair, t, h2, h, accT[h2]))
                        dve_load += 600.0

            while pvq:
                pvq.pop(0)[1]()
            while fin_q:
                fin_q.pop(0)()

    nc.finalize()
    return nc


def get_program(qk_dt="bfloat16", pv_dt="bfloat16"):
    key = (qk_dt, pv_dt)
    if key not in _BUILT:
        _BUILT[key] = _build(qk_dt, pv_dt)
    return _BUILT[key]


def make_in_maps(q, k, v, pv_dt="bfloat16"):
    """Host-side input staging: bf16 cast + transposed/blocked layouts."""
    import ml_dtypes

    bf = ml_dtypes.bfloat16
    qf = np.asarray(q, dtype=np.float32).reshape(B * H, N, D)
    kf = np.asarray(k, dtype=np.float32).reshape(B * H, N, D)
    vf = np.asarray(v, dtype=np.float32).reshape(B * H, N, D)

    j = np.arange(128)[:, None]
    cc = np.arange(128)[None, :]
    mg1 = (cc >= j).astype(bf)  # [128, 128]
    mg = np.ascontiguousarray(np.broadcast_to(mg1[:, None, :], (128, 2, 128)))

    maps = []
    for c in range(NCORES):
        base = c * HPC
        qtp = np.zeros((NPAIRS, 128, N), dtype=bf)
        kt2 = np.empty((NPAIRS, 128, N), dtype=bf)
        vo = np.empty((HPC, 128, NB, 65), dtype=bf)
        for p in range(NPAIRS):
            hA, hB = base + 2 * p, base + 2 * p + 1
            qtp[p, 0:64, :] = qf[hA].T.astype(bf)
            qtp[p, 64:128, :] = qf[hB].T.astype(bf)
            kt2[p, 0:64, :] = kf[hA].T.astype(bf)
            kt2[p, 64:128, :] = kf[hB].T.astype(bf)
        for hh in range(HPC):
            h = base + hh
            # ek from the bf16-rounded k (matches the on-device numerics
            # the QK path sees), folded into [V*ek | ek] on host.
            kh = kf[h].astype(bf).astype(np.float32)  # [N, 64]
            ek = np.exp(-SCALE * np.sum(kh * kh, axis=-1))  # [N]
            ekb = ek.reshape(NB, 128, 1).transpose(1, 0, 2)  # [128, NB, 1]
            vh = vf[h].reshape(NB, 128, D).transpose(1, 0, 2)
            vo[hh, :, :, 0:64] = (vh * ekb).astype(bf)
            vo[hh, :, :, 64] = ekb[:, :, 0].astype(bf)
        maps.append(
            {
                "qtp": qtp,
                "kt2": np.ascontiguousarray(kt2),
                "vo": vo,
                "mg": mg,
            }
        )
    return maps


def kernel(q, k, v):
    from concourse.bass_utils import run_bass_kernel_spmd

    nc = get_program()
    maps = make_in_maps(q, k, v)
    res = run_bass_kernel_spmd(nc, maps, list(range(NCORES)))
    out = np.concatenate(
        [res.results[c]["out"] for c in range(NCORES)], axis=0
    )  # [B*H, N, 64]
    return np.ascontiguousarray(out).reshape(B, H, N, D)
